# revision 7
# baseline (speedup 1.0000x reference)
"""Trainium2 Bass kernel for nn_Conv2d: x[32,128,56,56] * W[256,128,3,3] + b -> [32,256,56,56].

Stride 1, padding 1, dilation 1. Data-parallel over batch across 8 NeuronCores
(4 images per core, no collectives). Per core the conv is one accumulation
group of 9 matmuls per 8-row output tile (one per kernel tap):
PSUM[cout_chunk=128, 8*56] += matmul(lhsT=Wt[tap][cin, cout_chunk],
rhs=shifted window of the zero-padded input row-block [cin=128, 10, 58]).
Bias is fused into the PSUM->SBUF drain on the scalar engine.

Matmuls run in bf16 (1 PE cycle/row vs 4 for exact fp32; enables fast weight
load). PSUM accumulation and the output stay fp32; measured absmax rel err is
~2e-3 vs the fp32 reference.

Row-tiles are processed in SLABS of two (plus one 8-row single per image,
56 = 8 + 3*16): the two 9-matmul groups of a slab accumulate into the two
halves of one 2-bank PSUM tile and drain with a single scalar activation.
This halves the activation count and, with paired input DMAs and combined
output DMAs, cuts the number of cross-engine semaphores the framework must
reset in its end-of-kernel epilogue (the epilogue serially clears every
allocated semaphore at ~115ns each, so semaphore count is directly wall
clock).

DMA flow is just-in-time: slab input DMAs are interleaved with output DMAs
inside the main loop (2 slabs of prefetch) instead of bulk-issued up front.
The Sync queue triggers DMAs in order through an 8-slot completion window,
so bulk-issuing all input tiles parks every output DMA behind ~8 MB of
input traffic -> output buffers never recycle -> PSUM fills -> the PE
stalls mid-run and the HAM clock-gate re-throttles it. Interleaved issue
keeps the PE streaming continuously.

A 9-matmul warm-up group on zeroed SBUF (result never read) runs during the
initial DMA wait so the HAM activity monitor has the PE at full clock
(2.4 GHz, not the cold 1.2 GHz) before the first real matmul.

Self-contained: hardcodes shapes; host-side pre-pads/retiles x and
pre-transposes W so every device DMA is contiguous.
"""

import numpy as np

B, CIN, H, W_ = 32, 128, 56, 56
COUT, KH, KW = 256, 3, 3
NCORES = 8
BPC = B // NCORES          # images per core
R = 8                      # output rows per matmul group -> free dim R*56 = 448
NT = H // R                # row tiles per image (7: one single + three pairs)
NPAIR = 3                  # row-tile pairs per image
HP, WP = H + 2, W_ + 2     # padded 58x58
NCH = COUT // 128          # cout chunks (2)
NPIX = R * W_              # 448
PBANK = 512                # fp32 slots per PSUM bank

MM_DTYPE = "bfloat16"
PF_SLABS = 2               # slabs of input loaded ahead of consumption

_cache = {}


# Per image: which row-tile ships alone, and which pairs ship together.
# Image 0 leads with its single so the very first input DMA is small (the
# first matmul gates on it); later images trail with it so the final output
# DMA is small too.
def _slab_plan(n):
    if n == 0:
        return [("s", 0)] + [("p", k) for k in range(NPAIR)]
    return [("p", k) for k in range(NPAIR)] + [("s", NT - 1)]


def _pair_hts(n, k):
    return (1 + 2 * k, 2 + 2 * k) if n == 0 else (2 * k, 1 + 2 * k)


def _np_mm_dtype():
    if MM_DTYPE == "bfloat16":
        import ml_dtypes

        return ml_dtypes.bfloat16
    return np.float32


def _build(mm_dtype_name):
    import concourse.mybir as mybir
    import concourse.tile as tile
    from concourse import bacc

    dt = mybir.dt
    mmdt = getattr(dt, mm_dtype_name)

    nc = bacc.Bacc("TRN2", target_bir_lowering=False, debug=False)

    # x arrives host-pre-padded per row-tile (zero border baked in, halo rows
    # duplicated) so every x DMA is one fully contiguous copy and the kernel
    # needs no memsets: xp_d holds the row-tile pairs, x1_d the per-image
    # singles.
    xp_d = nc.dram_tensor(
        "xp", [BPC, NPAIR, CIN, 2, R + 2, WP], mmdt, kind="ExternalInput"
    )
    x1_d = nc.dram_tensor("x1", [BPC, CIN, R + 2, WP], mmdt, kind="ExternalInput")
    # [chunk, cin, tap, cout_slice]: one contiguous DMA per cout chunk
    wt_d = nc.dram_tensor("wt", [NCH, CIN, KH * KW, 128], mmdt, kind="ExternalInput")
    b_d = nc.dram_tensor("bias", [128, NCH], dt.float32, kind="ExternalInput")
    # Output laid out [image, cout%128 (partition), cout//128, h, w] so both
    # cout chunks of one slab go out in a single DMA; host untangles.
    o_d = nc.dram_tensor(
        "out", [BPC, 128, NCH, H, W_], dt.float32, kind="ExternalOutput"
    )

    slabs = []  # (n, kind, k_or_ht, r0)
    for n in range(BPC):
        r0 = 0
        for kind, k in _slab_plan(n):
            rows = R if kind == "s" else 2 * R
            slabs.append((n, kind, k, r0))
            r0 += rows
    NSLAB = len(slabs)

    with tile.TileContext(nc) as tc:
        with (
            tc.tile_pool(name="const", bufs=1) as const_pool,
            tc.tile_pool(name="xin", bufs=1) as xin_pool,
            tc.tile_pool(name="outp", bufs=3) as out_pool,
            tc.tile_pool(name="psum", bufs=3, space="PSUM") as psum_pool,
        ):
            xt = []

            def load_slab(i):
                n, kind, k, _ = slabs[i]
                if kind == "s":
                    t = xin_pool.tile([CIN, R + 2, WP], mmdt, tag="xs", bufs=2)
                    nc.sync.dma_start(t[:], x1_d[n])
                else:
                    t = xin_pool.tile([CIN, 2, R + 2, WP], mmdt, tag="xp", bufs=3)
                    nc.sync.dma_start(t[:], xp_d[n, k])
                xt.append(t)

            # PE clock warm-up: one 9-matmul group on zeroed SBUF during the
            # initial DMA wait brings the HAM clock gate to full rate before
            # the first real matmul.
            zw_t = const_pool.tile([CIN, 128], mmdt)
            nc.gpsimd.memset(zw_t[:], 0.0)
            zx_t = const_pool.tile([CIN, R, W_], mmdt)
            nc.gpsimd.memset(zx_t[:], 0.0)
            pw = psum_pool.tile([128, 2, PBANK], dt.float32, tag="pp")
            for i in range(9):
                nc.tensor.matmul(
                    pw[:, 0, 0:NPIX],
                    zw_t[:],
                    zx_t[:],
                    start=(i == 0),
                    stop=(i == 8),
                )

            # Critical path first: the first slab's input + chunk-0 weights.
            load_slab(0)
            w_t = const_pool.tile([CIN, NCH, KH * KW, 128], mmdt)
            nc.sync.dma_start(w_t[:, 0], wt_d[0])
            nc.sync.dma_start(w_t[:, 1], wt_d[1])
            b_t = const_pool.tile([128, NCH], dt.float32)
            nc.sync.dma_start(b_t[:], b_d[:])
            for i in range(1, PF_SLABS):
                load_slab(i)

            for i in range(NSLAB):
                n, kind, k, r0 = slabs[i]
                if i + PF_SLABS < NSLAB:
                    load_slab(i + PF_SLABS)
                t = xt[i]
                rows = R if kind == "s" else 2 * R
                ot = out_pool.tile([128, NCH, 2 * R, W_], dt.float32, tag="ot")
                for c in range(NCH):
                    p = psum_pool.tile([128, 2, PBANK], dt.float32, tag="pp")
                    for j in range(1 if kind == "s" else 2):
                        tj = t if kind == "s" else t[:, j]
                        for kh in range(KH):
                            for kw in range(KW):
                                pos = kh * KW + kw
                                nc.tensor.matmul(
                                    p[:, j, 0:NPIX],
                                    w_t[:, c, pos],
                                    tj[:, kh : kh + R, kw : kw + W_],
                                    start=(pos == 0),
                                    stop=(pos == KH * KW - 1),
                                )
                    if kind == "s":
                        nc.scalar.activation(
                            ot[:, c, 0:R],
                            p[:, 0, 0:NPIX],
                            mybir.ActivationFunctionType.Identity,
                            bias=b_t[:, c : c + 1],
                        )
                    else:
                        nc.scalar.activation(
                            ot[:, c],
                            p[:, :, 0:NPIX],
                            mybir.ActivationFunctionType.Identity,
                            bias=b_t[:, c : c + 1],
                        )
                    if i == NSLAB - 1:
                        # Tail latency: ship each chunk of the final slab as
                        # soon as its drain finishes instead of waiting for
                        # both.
                        nc.sync.dma_start(
                            o_d[n, :, c, r0 : r0 + rows, :],
                            ot[:, c, 0:rows],
                        )
                if i < NSLAB - 1:
                    nc.sync.dma_start(
                        o_d[n, :, :, r0 : r0 + rows, :],
                        ot[:, :, 0:rows],
                    )

    nc.compile()
    return nc


def _make_in_maps(x, W, b):
    mdt = _np_mm_dtype()
    x = np.asarray(x, dtype=np.float32)
    W = np.asarray(W, dtype=np.float32)
    b = np.asarray(b, dtype=np.float32)

    # Pre-pad and re-tile x: row-tile ht holds padded rows ht*R..ht*R+R+1
    # (zero border baked in), then split into per-image pairs + single.
    xpad = np.zeros((B, CIN, HP, WP), dtype=mdt)
    xpad[:, :, 1 : H + 1, 1 : W_ + 1] = x.astype(mdt)
    xt = np.empty((B, NT, CIN, R + 2, WP), dtype=mdt)
    for ht in range(NT):
        xt[:, ht] = xpad[:, :, ht * R : ht * R + R + 2, :]

    xp = np.empty((B, NPAIR, CIN, 2, R + 2, WP), dtype=mdt)
    x1 = np.empty((B, CIN, R + 2, WP), dtype=mdt)
    for bidx in range(B):
        n = bidx % BPC
        x1[bidx] = xt[bidx, 0 if n == 0 else NT - 1]
        for k in range(NPAIR):
            h0, h1 = _pair_hts(n, k)
            xp[bidx, k, :, 0] = xt[bidx, h0]
            xp[bidx, k, :, 1] = xt[bidx, h1]

    # [cout, cin, kh, kw] -> [cout_chunk, cin, kh*kw, cout_slice], contiguous
    wt = np.ascontiguousarray(
        W.reshape(NCH, 128, CIN, KH * KW).transpose(0, 2, 3, 1)
    ).astype(mdt)
    bh = np.ascontiguousarray(b.reshape(NCH, 128).T)

    return [
        {
            "xp": xp[core * BPC : (core + 1) * BPC],
            "x1": x1[core * BPC : (core + 1) * BPC],
            "wt": wt,
            "bias": bh,
        }
        for core in range(NCORES)
    ]


def kernel(x, W, b):
    from concourse.bass_utils import run_bass_kernel_spmd

    if MM_DTYPE not in _cache:
        _cache[MM_DTYPE] = _build(MM_DTYPE)
    nc = _cache[MM_DTYPE]

    in_maps = _make_in_maps(x, W, b)
    try:
        res = run_bass_kernel_spmd(nc, in_maps, list(range(NCORES))).results
    except Exception:
        # A prior session can leave the accelerator in a transient
        # unrecoverable state; one retry after re-init clears it.
        import time

        time.sleep(15)
        res = run_bass_kernel_spmd(nc, in_maps, list(range(NCORES))).results
    # [BPC, 128, NCH, H, W] -> [BPC, NCH*128, H, W]
    outs = [
        res[i]["out"].transpose(0, 2, 1, 3, 4).reshape(BPC, COUT, H, W_)
        for i in range(NCORES)
    ]
    return np.concatenate(outs, axis=0)


# revision 8
# speedup vs baseline: 1.2016x; 1.2016x over previous
"""Trainium2 Bass kernel for nn_Conv2d: x[32,128,56,56] * W[256,128,3,3] + b -> [32,256,56,56].

Stride 1, padding 1, dilation 1. Data-parallel over batch across 8 NeuronCores
(4 images per core, no collectives). Per core the conv is one accumulation
group of 9 matmuls per output tile (one per kernel tap):
PSUM[cout_chunk=128, R*56] += matmul(lhsT=Wt[tap][cin, cout_chunk],
rhs=shifted window of the zero-padded input row-block [cin=128, R+2, 58]).
Bias is fused into the PSUM->SBUF drain on the scalar engine.

Matmuls run in bf16 (1 PE cycle/row vs 4 for exact fp32; enables fast weight
load, so the per-matmul weight switch hides under the previous matmul's
streaming). PSUM accumulation and the output stay fp32; measured absmax rel
err is ~2e-3 vs the fp32 reference. The PSUM output AP is kept in its
natural [128, R, 56] shape — flattening it to [128, 448] measurably slows
every matmul by ~40ns.

DMA flow is just-in-time: x row-tile DMAs are interleaved with the output
DMAs inside the main loop (prefetch depth 5) instead of bulk-issued up
front. The Sync queue triggers DMAs in order through an 8-slot completion
window, so bulk-issuing all 28 input tiles parks every output DMA behind
~8 MB of input traffic -> output SBUF buffers never recycle -> PSUM fills
-> the PE stalls mid-run and the HAM clock-gate re-throttles it (measured
9 us stall + 10 us at half clock). Interleaved issue keeps the PE streaming
continuously.

A 9-matmul warm-up group on zeroed SBUF (result never read) runs during the
initial DMA wait so the HAM activity monitor has the PE at full clock
(2.4 GHz, not the cold 1.2 GHz) before the first real matmul.

Self-contained: hardcodes shapes; host-side pre-pads/retiles x and
pre-transposes W so every device DMA is contiguous.
"""

import numpy as np

B, CIN, H, W_ = 32, 128, 56, 56
COUT, KH, KW = 256, 3, 3
NCORES = 8
BPC = B // NCORES          # images per core
R = 8                      # output rows per tile -> matmul free dim R*56 = 448
NT = H // R                # row tiles per image
NTILE = BPC * NT
HP, WP = H + 2, W_ + 2     # padded 58x58
NCH = COUT // 128          # cout chunks (2)

MM_DTYPE = "bfloat16"
XBUFS = 6                  # x-tile ring depth
PREFETCH = 5               # x tiles loaded ahead of consumption

_cache = {}


def _np_mm_dtype():
    if MM_DTYPE == "bfloat16":
        import ml_dtypes

        return ml_dtypes.bfloat16
    return np.float32


def _build(mm_dtype_name):
    import concourse.mybir as mybir
    import concourse.tile as tile
    from concourse import bacc

    dt = mybir.dt
    mmdt = getattr(dt, mm_dtype_name)

    nc = bacc.Bacc("TRN2", target_bir_lowering=False, debug=False)

    # x arrives host-pre-padded per row-tile: [image, row_tile, cin, R+2, 58]
    # (zero border baked in, halo rows duplicated) so every x DMA is one
    # fully contiguous copy and the kernel needs no memsets.
    x_d = nc.dram_tensor(
        "x", [BPC, NT, CIN, R + 2, WP], mmdt, kind="ExternalInput"
    )
    # [chunk, cin, tap, cout_slice]: one contiguous DMA per cout chunk
    wt_d = nc.dram_tensor("wt", [NCH, CIN, KH * KW, 128], mmdt, kind="ExternalInput")
    b_d = nc.dram_tensor("bias", [128, NCH], dt.float32, kind="ExternalInput")
    # Output laid out [image, cout%128 (partition), cout//128, h, w] so both
    # cout chunks of one row-tile go out in a single DMA; host untangles.
    o_d = nc.dram_tensor(
        "out", [BPC, 128, NCH, H, W_], dt.float32, kind="ExternalOutput"
    )

    with tile.TileContext(nc) as tc:
        with (
            tc.tile_pool(name="const", bufs=1) as const_pool,
            tc.tile_pool(name="xin", bufs=XBUFS) as xin_pool,
            tc.tile_pool(name="outp", bufs=4) as out_pool,
            tc.tile_pool(name="psum", bufs=8, space="PSUM") as psum_pool,
        ):
            xt = []

            def load_x(idx):
                n, ht = divmod(idx, NT)
                t = xin_pool.tile([CIN, R + 2, WP], mmdt, tag="xt")
                nc.sync.dma_start(t[:], x_d[n, ht])
                xt.append(t)

            # PE clock warm-up: the HAM activity monitor keeps the PE at half
            # clock until it has been busy ~3.4us. One 9-matmul group on
            # zeroed SBUF (result never read) during the initial DMA wait
            # brings it to full rate before the first real matmul.
            zw_t = const_pool.tile([CIN, 128], mmdt)
            nc.gpsimd.memset(zw_t[:], 0.0)
            zx_t = const_pool.tile([CIN, R, W_], mmdt)
            nc.gpsimd.memset(zx_t[:], 0.0)
            pw = psum_pool.tile([128, R, W_], dt.float32, tag="ps")
            for i in range(9):
                nc.tensor.matmul(
                    pw[:],
                    zw_t[:],
                    zx_t[:],
                    start=(i == 0),
                    stop=(i == 8),
                )

            # Critical path first: the first x tile (the startup gater), then
            # tap-0 of chunk-0 weights (all the first matmul needs), then the
            # rest of the constants and the prefetch window.
            load_x(0)
            w_t = const_pool.tile([CIN, NCH, KH * KW, 128], mmdt)
            nc.sync.dma_start(w_t[:, 0, 0], wt_d[0, :, 0])
            nc.sync.dma_start(w_t[:, 0, 1:], wt_d[0, :, 1:])
            nc.sync.dma_start(w_t[:, 1], wt_d[1])
            b_t = const_pool.tile([128, NCH], dt.float32)
            nc.sync.dma_start(b_t[:], b_d[:])
            for i in range(1, PREFETCH):
                load_x(i)

            for idx in range(NTILE):
                n, ht = divmod(idx, NT)
                if idx + PREFETCH < NTILE:
                    load_x(idx + PREFETCH)
                t = xt[idx]
                ot = out_pool.tile([128, NCH, R, W_], dt.float32, tag="ot")
                for c in range(NCH):
                    p = psum_pool.tile([128, R, W_], dt.float32, tag="ps")
                    for kh in range(KH):
                        for kw in range(KW):
                            pos = kh * KW + kw
                            nc.tensor.matmul(
                                p[:],
                                w_t[:, c, pos],
                                t[:, kh : kh + R, kw : kw + W_],
                                start=(pos == 0),
                                stop=(pos == KH * KW - 1),
                            )
                    nc.scalar.activation(
                        ot[:, c],
                        p[:],
                        mybir.ActivationFunctionType.Identity,
                        bias=b_t[:, c : c + 1],
                    )
                    if idx == NTILE - 1:
                        # Tail latency: ship each chunk of the final tile as
                        # soon as its drain finishes instead of waiting for
                        # both.
                        nc.sync.dma_start(
                            o_d[n, :, c, ht * R : ht * R + R, :],
                            ot[:, c],
                        )
                if idx < NTILE - 1:
                    nc.sync.dma_start(
                        o_d[n, :, :, ht * R : ht * R + R, :],
                        ot[:],
                    )

    nc.compile()
    return nc


def _make_in_maps(x, W, b):
    mdt = _np_mm_dtype()
    x = np.asarray(x, dtype=np.float32)
    W = np.asarray(W, dtype=np.float32)
    b = np.asarray(b, dtype=np.float32)

    # Pre-pad and re-tile x: [B, CIN, 56, 56] -> [B, NT, CIN, R+2, 58] where
    # row-tile ht holds padded rows h0..h0+R+1 (zero border baked in).
    xpad = np.zeros((B, CIN, HP, WP), dtype=mdt)
    xpad[:, :, 1 : H + 1, 1 : W_ + 1] = x.astype(mdt)
    xt = np.empty((B, NT, CIN, R + 2, WP), dtype=mdt)
    for ht in range(NT):
        xt[:, ht] = xpad[:, :, ht * R : ht * R + R + 2, :]

    # [cout, cin, kh, kw] -> [cout_chunk, cin, kh*kw, cout_slice], contiguous
    wt = np.ascontiguousarray(
        W.reshape(NCH, 128, CIN, KH * KW).transpose(0, 2, 3, 1)
    ).astype(mdt)
    bh = np.ascontiguousarray(b.reshape(NCH, 128).T)

    return [
        {
            "x": xt[core * BPC : (core + 1) * BPC],
            "wt": wt,
            "bias": bh,
        }
        for core in range(NCORES)
    ]


def kernel(x, W, b):
    from concourse.bass_utils import run_bass_kernel_spmd

    if MM_DTYPE not in _cache:
        _cache[MM_DTYPE] = _build(MM_DTYPE)
    nc = _cache[MM_DTYPE]

    in_maps = _make_in_maps(x, W, b)
    try:
        res = run_bass_kernel_spmd(nc, in_maps, list(range(NCORES))).results
    except Exception:
        # A prior session can leave the accelerator in a transient
        # unrecoverable state; one retry after re-init clears it.
        import time

        time.sleep(15)
        res = run_bass_kernel_spmd(nc, in_maps, list(range(NCORES))).results
    # [BPC, 128, NCH, H, W] -> [BPC, NCH*128, H, W]
    outs = [
        res[i]["out"].transpose(0, 2, 1, 3, 4).reshape(BPC, COUT, H, W_)
        for i in range(NCORES)
    ]
    return np.concatenate(outs, axis=0)
